# revision 19
# baseline (speedup 1.0000x reference)
"""Location-sensitive attention kernel for Trainium2 (8 NeuronCores).

Data-parallel over batch: B=64 -> 8 batches per core.

Per-core pipeline (per batch b):
  1. DMA encoder slice [128t, 16, 512e] (natural layout, t on partitions)
  2. PE-transpose 128x128 tiles -> enc_T [e on partitions, t free]
  3. Projection X_T[a, t] = W_enc.T @ enc.T + G.T @ P + dec_proj (PSUM accum):
       - 4 matmuls over e-chunks (lhsT = W_enc chunk, rhs = enc_T chunk)
       - 1 matmul conv-as-matmul: G[k, a] = sum_nf conv_w[nf, k] W_loc[nf, a],
         P[k, t] = prev_pad[t + k] (im2col via overlapping-window DMA)
       - dec_proj folded into tanh bias (per-partition bias on ACT)
  4. tanh on ACT -> tanhX [a, t]
  5. energy[t] = v . tanhX[:, t] via matmuls -> PSUM columns [128t-chunk, 1]
  6. masked exp (no max subtraction: |energy| <= ||v||_1, exp can't overflow),
     row-sums via ACT accum, total Z via ones-matmul, 1/Z broadcast via matmul
  7. context = sum_t attn[t] enc[t, :] via accumulated M=1 matmuls on natural tiles

All static params are packed host-side into one [128, PW] tensor (single DMA,
single semaphore) because matmul/DMA instructions only support one sync wait.
PSUM->SBUF copies are pinned to fixed engines (enc_T -> DVE, rest -> ACT) for
the same reason.
"""

import os
import sys
import numpy as np

sys.path.insert(0, "/opt/trn_rl_repo")

B, T, E, D, A, NF, KW = 64, 2000, 512, 1024, 256, 32, 31
NCORES = 8
B_L = B // NCORES          # 8 batches per core
PAD = (KW - 1) // 2        # 15
NI = 16                    # number of 128-row t-chunks (last has 80 rows)
TAIL = T - 15 * 128        # 80
NTC, TC = 4, 500           # projection moving-dim chunks
NE = E // 128              # 4
NA = A // 128              # 2
ND = D // 128              # 8

# packed-params column offsets
_O_ID = 0
_O_WENC = _O_ID + 128
_O_V = _O_WENC + NE * A
_O_CW = _O_V + NA
_O_WLOC = _O_CW + KW
_O_WDEC = _O_WLOC + A
PW = _O_WDEC + ND * A

_CACHE = {}


def _build(use_f32r: bool):
    import concourse.bass as bass
    import concourse.bacc as bacc
    import concourse.tile as tile
    from concourse import mybir
    from concourse.ap import AP
    from contextlib import ExitStack

    F32 = mybir.dt.float32
    F32R = mybir.dt.float32r
    I32 = mybir.dt.int32
    AF = mybir.ActivationFunctionType

    FR = F32R if use_f32r else F32

    def as32(ap):
        # plain-fp32 view for tiny matmuls where f32r fails the ISA check
        return ap.bitcast(F32) if use_f32r else ap

    nc = bacc.Bacc("TRN2", target_bir_lowering=False, debug=False,
                   num_devices=NCORES)

    enc_d = nc.dram_tensor("enc", [B_L, T, E], FR, kind="ExternalInput").ap()
    dec_d = nc.dram_tensor("dec", [B_L, D], FR, kind="ExternalInput").ap()
    prev_d = nc.dram_tensor("prev", [B_L, T + 2 * PAD], FR,
                            kind="ExternalInput").ap()
    mask_d = nc.dram_tensor("mask", [128, B_L * NI], I32,
                            kind="ExternalInput").ap()
    par_d = nc.dram_tensor("par", [128, PW], FR, kind="ExternalInput").ap()

    ctx_d = nc.dram_tensor("ctx_out", [B_L, E], F32, kind="ExternalOutput").ap()
    attn_d = nc.dram_tensor("attn_out", [B_L, T], F32, kind="ExternalOutput").ap()

    with tile.TileContext(nc) as tc, ExitStack() as ctx:
        const = ctx.enter_context(tc.tile_pool(name="const", bufs=1))
        enc_pool = ctx.enter_context(tc.tile_pool(name="enc", bufs=2))
        encT_pool = ctx.enter_context(tc.tile_pool(name="encT", bufs=1))
        tanh_pool = ctx.enter_context(tc.tile_pool(name="tanh", bufs=2))
        p_pool = ctx.enter_context(tc.tile_pool(name="p", bufs=2))
        cols_pool = ctx.enter_context(tc.tile_pool(name="cols", bufs=2))
        row_pool = ctx.enter_context(tc.tile_pool(name="row", bufs=1))

        # ---- constants (single packed DMA) ----
        params = const.tile([128, PW], FR)
        nc.sync.dma_start(params[:], par_d)
        ident_sb = params[:, _O_ID:_O_ID + 128]
        wenc_sb = params[:, _O_WENC:_O_WENC + NE * A].rearrange(
            "p (j a) -> p j a", j=NE)
        v_sb = params[:, _O_V:_O_V + NA]
        cw_sb = params[:NF, _O_CW:_O_CW + KW]
        wloc_sb = params[:NF, _O_WLOC:_O_WLOC + A]
        wdec_sb = params[:, _O_WDEC:_O_WDEC + ND * A].rearrange(
            "p (j a) -> p j a", j=ND)

        mask_all = const.tile([128, B_L * NI], I32)
        nc.sync.dma_start(mask_all[:], mask_d)
        # absorb the mask DMA wait on DVE early so later DVE ops carry <=1 wait
        mask_touch = const.tile([1, 1], I32)
        nc.vector.tensor_copy(out=mask_touch[:], in_=mask_all[0:1, 0:1])

        g_sb = const.tile([KW, A], FR)
        dec_sb = const.tile([128, NA * B_L], F32)
        ones_col = const.tile([128, 1], F32)
        nc.vector.memset(ones_col[:], 1.0)
        ones_row = const.tile([1, 128], F32)
        nc.vector.memset(ones_row[:], 1.0)
        neg_sb = const.tile([128, NI], F32)
        nc.vector.memset(neg_sb[:], -1e9)
        prevpad = const.tile([B_L, T + 2 * PAD], FR)
        nc.sync.dma_start(prevpad[:], prev_d)

        ps_xt = ctx.enter_context(tc.tile_pool(name="ps_xt", bufs=2, space="PSUM"))
        ps_tp = ctx.enter_context(tc.tile_pool(name="ps_tp", bufs=3, space="PSUM"))
        ps_e = ctx.enter_context(tc.tile_pool(name="ps_e", bufs=1, space="PSUM"))
        ps_g = ctx.enter_context(tc.tile_pool(name="ps_g", bufs=1, space="PSUM"))
        # dec (setup), z and ctx (main) share one bank via a common tag
        ps_cd = ctx.enter_context(tc.tile_pool(name="ps_cd", bufs=1, space="PSUM"))

        with tc.tile_pool(name="setup", bufs=1) as sp:
            # G[k, a] = sum_nf conv_w[nf, k] * W_loc[nf, a]
            # (copy pinned to DVE: later proj matmuls' encT waits cover it)
            g_ps = ps_g.tile([KW, A], F32, tag="g")
            nc.tensor.matmul(g_ps[:], cw_sb[:], wloc_sb[:],
                             start=True, stop=True)
            nc.vector.tensor_copy(out=g_sb[:], in_=g_ps[:])

            # dec_proj[a, b] = sum_d W_dec[d, a] * dec[b, d]
            sT_sb = sp.tile([128, ND, B_L], FR)
            for j in range(ND):
                nc.sync.dma_start(
                    sT_sb[:, j, :],
                    dec_d[:, j * 128:(j + 1) * 128].rearrange("b p -> p b"))
            for ac in range(NA):
                d_ps = ps_cd.tile([128, B_L], F32, tag="cd")
                for j in range(ND):
                    nc.tensor.matmul(
                        d_ps[:], as32(wdec_sb[:, j, ac * 128:(ac + 1) * 128]),
                        as32(sT_sb[:, j, :]),
                        start=(j == 0), stop=(j == ND - 1))
                nc.scalar.copy(dec_sb[:, ac * B_L:(ac + 1) * B_L], d_ps[:])

        ctxrow = row_pool.tile([1, B_L * E], F32)

        # ---- main per-batch pipeline ----
        for b in range(B_L):
            nat = enc_pool.tile([128, NI, E], FR, tag="nat")
            nc.sync.dma_start(
                nat[:, :NI - 1, :],
                enc_d[b, :15 * 128, :].rearrange("(i p) e -> p i e", p=128))
            nc.sync.dma_start(nat[:TAIL, NI - 1, :], enc_d[b, 15 * 128:, :])

            # conv im2col: P[k, t] = prev_pad[b, t + k]
            p_b = p_pool.tile([KW, T], FR, tag="p")
            src_row = prevpad[b:b + 1, 0:1]
            p_src = AP(src_row.tensor, src_row.offset,
                       [[T + 2 * PAD, 1], [1, KW], [1, T]])
            nc.sync.dma_start(p_b[:], p_src)

            # transpose enc tiles: enc_T[j][e, t]; 4 transposes share one
            # PSUM bank so each PSUM->SBUF copy moves [128, 512]
            encT = [encT_pool.tile([128, T], FR, tag=f"encT{j}",
                                   name=f"encT{j}_{b}")
                    for j in range(NE)]
            for j in range(NE):
                for i0 in range(0, NI, 4):
                    tp = ps_tp.tile([128, 512], FR, tag="tp")
                    w = 0
                    for i in range(i0, i0 + 4):
                        cnt = 128 if i < NI - 1 else TAIL
                        nc.tensor.matmul(
                            tp[:, w:w + cnt],
                            nat[:cnt, i, j * 128:(j + 1) * 128],
                            ident_sb[:cnt, :cnt],
                            is_transpose=True, start=True, stop=True)
                        w += cnt
                    nc.any.tensor_copy(
                        out=encT[j][:, i0 * 128:i0 * 128 + w],
                        in_=tp[:, :w])

            # projection + tanh
            tanhX = [tanh_pool.tile([128, T], FR, tag=f"tanhX{ac}",
                                    name=f"tanhX{ac}_{b}")
                     for ac in range(NA)]
            for ac in range(NA):
                for t in range(NTC):
                    xt = ps_xt.tile([128, TC], F32, tag="xt")
                    tsl = slice(t * TC, (t + 1) * TC)
                    for j in range(NE):
                        nc.tensor.matmul(
                            xt[:], wenc_sb[:, j, ac * 128:(ac + 1) * 128],
                            encT[j][:, tsl],
                            start=(j == 0), stop=False)
                    nc.tensor.matmul(
                        xt[:], g_sb[:, ac * 128:(ac + 1) * 128],
                        p_b[:, tsl], start=False, stop=True)
                    nc.scalar.activation(
                        tanhX[ac][:, tsl], xt[:], AF.Tanh,
                        bias=dec_sb[:, ac * B_L + b:ac * B_L + b + 1], scale=1.0)

            # energy columns [128, NI]
            e_ps = ps_e.tile([128, NI], F32, tag="eps")
            for i in range(NI):
                cnt = 128 if i < NI - 1 else TAIL
                for ac in range(NA):
                    nc.tensor.matmul(
                        e_ps[:cnt, i:i + 1],
                        as32(tanhX[ac][:, i * 128:i * 128 + cnt]),
                        as32(v_sb[:, ac:ac + 1]),
                        start=(ac == 0), stop=(ac == NA - 1))
            e_sb = cols_pool.tile([128, NI], F32, tag="e_sb")
            nc.scalar.copy(e_sb[:], e_ps[:])

            # mask -> -1e9 (also covers garbage tail rows of the last chunk)
            e_m = cols_pool.tile([128, NI], F32, tag="em")
            nc.vector.select(e_m[:], mask_all[:, b * NI:(b + 1) * NI],
                             e_sb[:], neg_sb[:])

            # exp (no max subtraction; energies bounded by ||v||_1)
            exp_cols = cols_pool.tile([128, NI], F32, tag="exp")
            rowsum = cols_pool.tile([128, 1], F32, tag="rowsum")
            nc.scalar.activation(exp_cols[:], e_m[:], AF.Exp,
                                 accum_out=rowsum[:])

            # Z = sum over partitions, r = 1/Z broadcast to all partitions
            z_ps = ps_cd.tile([1, 1], F32, tag="cd", name=f"z_ps_{b}")
            nc.tensor.matmul(z_ps[:], ones_col[:], rowsum[:],
                             start=True, stop=True)
            z_sb = cols_pool.tile([1, 1], F32, tag="z_sb")
            nc.scalar.copy(z_sb[:], z_ps[:])
            r_sb = cols_pool.tile([1, 1], F32, tag="r_sb")
            nc.vector.reciprocal(r_sb[:], z_sb[:])
            rb_ps = ps_g.tile([128, 1], F32, tag="g", name=f"rb_ps_{b}")
            nc.tensor.matmul(rb_ps[:], ones_row[:], r_sb[:],
                             start=True, stop=True)
            rb_sb = cols_pool.tile([128, 1], F32, tag="rb_sb")
            nc.vector.tensor_copy(out=rb_sb[:], in_=rb_ps[:])

            attn_cols = cols_pool.tile([128, NI], F32, tag="attn")
            nc.vector.tensor_scalar_mul(attn_cols[:], exp_cols[:], rb_sb[:])
            attn_cols_r = cols_pool.tile([128, NI], FR, tag="attnr")
            if use_f32r:
                nc.vector.tensor_scalar_mul(attn_cols_r[:], exp_cols[:], rb_sb[:])
            else:
                attn_cols_r = attn_cols

            # context
            ctx_ps = ps_cd.tile([1, E], F32, tag="cd")
            for i in range(NI):
                cnt = 128 if i < NI - 1 else TAIL
                nc.tensor.matmul(ctx_ps[:], attn_cols_r[:cnt, i:i + 1],
                                 nat[:cnt, i, :],
                                 start=(i == 0), stop=(i == NI - 1))
            nc.scalar.copy(ctxrow[:, b * E:(b + 1) * E], ctx_ps[:])

            # attention output
            nc.sync.dma_start(
                attn_d[b, :15 * 128].rearrange("(c p) -> p c", p=128),
                attn_cols[:, :NI - 1])
            nc.sync.dma_start(attn_d[b, 15 * 128:], attn_cols[:TAIL, NI - 1:NI])

        nc.sync.dma_start(ctx_d.rearrange("b e -> (b e)"), ctxrow[:])

    nc.compile()
    return nc


def _get_nc(use_f32r: bool):
    key = ("nc", use_f32r)
    if key not in _CACHE:
        _CACHE[key] = _build(use_f32r)
    return _CACHE[key]


def _pack_params(W_enc, W_dec, W_loc, conv_w, v_w):
    par = np.zeros((128, PW), np.float32)
    par[:, _O_ID:_O_ID + 128] = np.eye(128, dtype=np.float32)
    par[:, _O_WENC:_O_WENC + NE * A] = (
        W_enc.reshape(NE, 128, A).transpose(1, 0, 2).reshape(128, NE * A))
    par[:, _O_V:_O_V + NA] = v_w.reshape(NA, 128).T
    par[:NF, _O_CW:_O_CW + KW] = conv_w[:, 0, :]
    par[:NF, _O_WLOC:_O_WLOC + A] = W_loc
    par[:, _O_WDEC:_O_WDEC + ND * A] = (
        W_dec.reshape(ND, 128, A).transpose(1, 0, 2).reshape(128, ND * A))
    return par


def _make_in_maps(dec, enc, prev, mask, conv_w, W_enc, W_dec, W_loc, v_w):
    par = _pack_params(np.asarray(W_enc, np.float32),
                       np.asarray(W_dec, np.float32),
                       np.asarray(W_loc, np.float32),
                       np.asarray(conv_w, np.float32),
                       np.asarray(v_w, np.float32))
    mask_pad = np.pad(mask, ((0, 0), (0, NI * 128 - T)))
    mask_cols = mask_pad.reshape(B, NI, 128).transpose(2, 0, 1)
    in_maps = []
    for c in range(NCORES):
        s = slice(c * B_L, (c + 1) * B_L)
        in_maps.append({
            "enc": np.ascontiguousarray(enc[s]),
            "dec": np.ascontiguousarray(dec[s]),
            "prev": np.pad(prev[s], ((0, 0), (PAD, PAD))),
            "mask": np.ascontiguousarray(
                mask_cols[:, s, :].reshape(128, B_L * NI)),
            "par": par,
        })
    return in_maps


def bench(decoder_state, encoder_outputs, prev_attention_weights, encoder_mask,
          conv_w, W_enc, W_dec, W_loc, v_w, reps=20, _f32r=None):
    """Steady-state wall time per launch with device-resident inputs.

    Upper bound on HW exec time (includes dispatch via the axon proxy)."""
    import time
    import jax
    import jax.numpy as jnp
    from jax.sharding import Mesh, PartitionSpec, NamedSharding
    from jax.experimental.shard_map import shard_map
    from concourse import bass2jax, mybir
    from concourse.bass2jax import _bass_exec_p, partition_id_tensor, \
        install_neuronx_cc_hook

    if _f32r is None:
        _f32r = os.environ.get("LSA_F32R", "1") == "1"
    nc = _get_nc(_f32r)
    install_neuronx_cc_hook()

    in_maps = _make_in_maps(
        np.asarray(decoder_state, np.float32),
        np.asarray(encoder_outputs, np.float32),
        np.asarray(prev_attention_weights, np.float32),
        np.asarray(encoder_mask, np.int32),
        conv_w, W_enc, W_dec, W_loc, v_w)

    partition_name = (nc.partition_id_tensor.name
                      if nc.partition_id_tensor else None)
    in_names, out_names, out_avals, zero_outs = [], [], [], []
    for alloc in nc.m.functions[0].allocations:
        if not isinstance(alloc, mybir.MemoryLocationSet):
            continue
        name = alloc.memorylocations[0].name
        if alloc.kind == "ExternalInput":
            if name != partition_name:
                in_names.append(name)
        elif alloc.kind == "ExternalOutput":
            shape = tuple(alloc.tensor_shape)
            dtype = mybir.dt.np(alloc.dtype)
            out_names.append(name)
            out_avals.append(jax.core.ShapedArray(shape, dtype))
            zero_outs.append(np.zeros(shape, dtype))
    n_params = len(in_names)
    n_outs = len(out_avals)
    in_names_all = in_names + out_names
    if partition_name is not None:
        in_names_all.append(partition_name)

    def _body(*args):
        operands = list(args)
        if partition_name is not None:
            operands.append(partition_id_tensor())
        return tuple(_bass_exec_p.bind(
            *operands, out_avals=tuple(out_avals),
            in_names=tuple(in_names_all), out_names=tuple(out_names),
            lowering_input_output_aliases=(),
            sim_require_finite=True, sim_require_nnan=True, nc=nc))

    devices = jax.devices()[:NCORES]
    mesh = Mesh(np.asarray(devices), ("core",))
    in_specs = (PartitionSpec("core"),) * (n_params + n_outs)
    out_specs = (PartitionSpec("core"),) * n_outs
    fn = jax.jit(shard_map(_body, mesh=mesh, in_specs=in_specs,
                           out_specs=out_specs, check_rep=False),
                 keep_unused=True)

    sh = NamedSharding(mesh, PartitionSpec("core"))
    dev_in = [jax.device_put(
        np.concatenate([np.asarray(in_maps[c][nm])[None] if False else
                        np.asarray(in_maps[c][nm])
                        for c in range(NCORES)], axis=0).reshape(
            (NCORES * np.asarray(in_maps[0][nm]).shape[0],) +
            np.asarray(in_maps[0][nm]).shape[1:]), sh)
        for nm in in_names]
    dev_zero = [jax.device_put(
        np.concatenate([z] * NCORES, axis=0), sh) for z in zero_outs]

    # warmup (compiles)
    r = fn(*dev_in, *dev_zero)
    jax.block_until_ready(r)
    times = []
    for _ in range(reps):
        t0 = time.perf_counter()
        r = fn(*dev_in, *dev_zero)
        jax.block_until_ready(r)
        times.append(time.perf_counter() - t0)
    times.sort()
    print(f"bench: min={times[0]*1e6:.0f}us median={times[len(times)//2]*1e6:.0f}us "
          f"max={times[-1]*1e6:.0f}us over {reps} reps")
    return times[0] * 1e9


def kernel(decoder_state, encoder_outputs, prev_attention_weights, encoder_mask,
           conv_w, W_enc, W_dec, W_loc, v_w, _trace=False, _f32r=None):
    from concourse import bass_utils

    if _f32r is None:
        _f32r = os.environ.get("LSA_F32R", "1") == "1"
    nc = _get_nc(_f32r)

    in_maps = _make_in_maps(
        np.asarray(decoder_state, np.float32),
        np.asarray(encoder_outputs, np.float32),
        np.asarray(prev_attention_weights, np.float32),
        np.asarray(encoder_mask, np.int32),
        conv_w, W_enc, W_dec, W_loc, v_w)

    res = bass_utils.run_bass_kernel_spmd(
        nc, in_maps, core_ids=list(range(NCORES)), trace=_trace)

    ctx = np.concatenate([r["ctx_out"] for r in res.results], axis=0)
    attn = np.concatenate([r["attn_out"] for r in res.results], axis=0)
    if _trace:
        kernel.last_exec_time_ns = res.exec_time_ns
        kernel.last_results = res
    return ctx, attn
